# revision 39
# baseline (speedup 1.0000x reference)
"""BiDAF attention Bass kernel for Trainium2 (8 NeuronCores, batch-parallel).

Takes FULL inputs (BS=32, MCL=1024, MQL=64, d=512), shards batch across the
8 cores (4 batches/core), runs one SPMD Bass kernel, gathers the full output
(32, 1024, 2048) float32.

v3: fp16 on device (host casts in/out). S^T via big-N matmuls; softmax stats
from PE-transposed S tiles; paired En transposes + row-packed A matmuls using
a partition-duplicated Hq; products split DVE (Hc*A from PSUM) / GpSimd
(Hc*U); q2c normalization folded into the tanh scale to shorten the chain;
all loads issued upfront on the SP HWDGE ring, stores on the ACT HWDGE ring.

Self-contained: only imports concourse (available on sys.path in the
container via sitecustomize).
"""
import sys

if "/opt/trn_rl_repo" not in sys.path:
    sys.path.insert(0, "/opt/trn_rl_repo")

from contextlib import ExitStack

import numpy as np

import concourse.bass as bass
import concourse.bacc as bacc
import concourse.tile as tile
from concourse import mybir

dt = mybir.dt
AF = mybir.ActivationFunctionType
ALU = mybir.AluOpType
AX = mybir.AxisListType

NCORES = 8
BS, MCL, MQL, D = 32, 1024, 64, 512
BPC = BS // NCORES          # batches per core
NT = MCL // 128             # c-tiles per batch
NK = D // 128               # contraction chunks
F32 = dt.float32
F16 = dt.float16
EXP_BIAS = -3.0             # constant shift inside c2q softmax (exact in softmax math)


def build_nc():
    nc = bacc.Bacc("TRN2", target_bir_lowering=False)
    hq_d = nc.dram_tensor("hq", [BPC, MQL, D], F16, kind="ExternalInput")
    hc_d = nc.dram_tensor("hc", [BPC, MCL, D], F16, kind="ExternalInput")
    w_d = nc.dram_tensor("w", [3 * D, 1], F32, kind="ExternalInput")
    id_d = nc.dram_tensor("idm", [128, 128], F16, kind="ExternalInput")
    out_d = nc.dram_tensor("out", [BPC, MCL, 4 * D], F16, kind="ExternalOutput")

    with tile.TileContext(nc) as tc, ExitStack() as ctx:
        const = ctx.enter_context(tc.tile_pool(name="const", bufs=1))
        sb = ctx.enter_context(tc.tile_pool(name="sb", bufs=2))
        sbc = ctx.enter_context(tc.tile_pool(name="sbc", bufs=4))
        sbo = ctx.enter_context(tc.tile_pool(name="sbo", bufs=3))
        ps = ctx.enter_context(tc.tile_pool(name="ps", bufs=1, space="PSUM"))
        psA = ctx.enter_context(tc.tile_pool(name="psA", bufs=2, space="PSUM"))
        psT = ctx.enter_context(tc.tile_pool(name="psT", bufs=2, space="PSUM"))
        psU = ctx.enter_context(tc.tile_pool(name="psU", bufs=1, space="PSUM"))

        # ---- constants ----
        ones_r = const.tile([1, 512], F16)
        nc.vector.memset(ones_r[:], 1.0)
        ones_col = const.tile([128, 1], F16)
        nc.vector.memset(ones_col[:], 1.0)
        bias_e = const.tile([128, 1], F32)
        nc.vector.memset(bias_e[:], EXP_BIAS)
        bias_0 = const.tile([128, 1], F32)
        nc.vector.memset(bias_0[:], 0.0)
        # dummy activation: pull the exp/tanh ACT table load to t~0
        warm = const.tile([128, 1], F16)
        nc.scalar.activation(warm[:], bias_0[:], AF.Exp, bias=bias_0[:],
                             scale=1.0)
        # W and hq on the scalar HWDGE ring (parallel to hc on the SP ring)
        wv = const.tile([128, 12], F32)
        nc.scalar.dma_start(wv[:], w_d.rearrange("(j p) o -> p (j o)", p=128))
        wv16 = const.tile([128, 12], F16)
        nc.vector.tensor_copy(wv16[:], wv[:])
        ident = const.tile([128, 128], F16)
        nc.sync.dma_start(ident[:], id_d[:])
        # hq duplicated on partitions 0-63 / 64-127: (128, BPC, 512)
        hq2 = const.tile([128, BPC, D], F16)
        nc.scalar.dma_start(hq2[0:MQL], hq_d.rearrange("b q d -> q b d"))
        nc.scalar.dma_start(hq2[MQL:128], hq_d.rearrange("b q d -> q b d"))
        hc0 = sbc.tile([128, NT, D], F16, tag="hc", name="hc0")
        nc.sync.dma_start(hc0[:], hc_d[0].rearrange("(p t) d -> p t d", p=128))

        def heater(n=1):
            # keep the PE pipeline streaming through transpose-heavy
            # stretches: standalone weight loads with no output and no deps
            # (the next real matmul reloads its own weights anyway).
            for _ in range(n):
                nc.tensor.ldweights(ident[:])

        # warm the PE clock during the initial DMA wait (~3.4us of real
        # matmul activity flips the clock gate from 1.2 to 2.4 GHz)
        heater(60)

        st = [dict() for _ in range(BPC)]   # per-batch live tiles

        # remaining context loads upfront (bufs=4 -> no reuse stalls)
        st[0]["hc_nat"] = hc0
        for b in range(1, BPC):
            hc_nat = sbc.tile([128, NT, D], F16, tag="hc")
            nc.sync.dma_start(hc_nat[:],
                              hc_d[b].rearrange("(p t) d -> p t d", p=128))
            st[b]["hc_nat"] = hc_nat

        # hq-only work (hqT -> stw, sq) hoisted out of the batch loop:
        # it runs in the head while the hc loads stream in.
        for b in range(BPC):
            v = st[b]
            hq_r = hq2[0:MQL, b, :]
            hqT_ps = psU.tile([128, NK, MQL], F16, tag="u")
            for k in range(NK):
                nc.tensor.transpose(
                    hqT_ps[:, k, :], hq_r[:, k * 128:(k + 1) * 128],
                    ident[0:MQL, 0:MQL])
            hqT_s = sb.tile([128, NK, MQL], F16, tag="hqT")
            nc.vector.tensor_copy(hqT_s[:], hqT_ps[:])
            stw = sb.tile([128, NK, MQL + 1], F16, tag="stw", bufs=4)
            for k in range(NK):
                nc.vector.tensor_scalar(
                    stw[:, k, 0:MQL], hqT_s[:, k, :],
                    wv[:, 8 + k, None], None, op0=ALU.mult)
            nc.vector.tensor_copy(stw[:, :, MQL], wv16[:, 0:NK])
            sq_ps = psU.tile([1, MQL], F32, tag="u")
            for k in range(NK):
                nc.tensor.matmul(sq_ps[:], wv16[:, 4 + k, None], hqT_s[:, k, :],
                                 start=(k == 0), stop=(k == NK - 1))
            sq_aug = sb.tile([1, MQL + 1], F16, tag="sq_aug", bufs=4)
            nc.vector.memset(sq_aug[:], 0.0)
            nc.vector.tensor_copy(sq_aug[0:1, 0:MQL], sq_ps[:])
            v["stw"], v["sq_aug"] = stw, sq_aug

        def s1(b):
            """Hc transposes, S^T matmuls, sbank transposes, q2c chain."""
            v = st[b]
            hc_nat = v["hc_nat"]
            stw, sq_aug = v["stw"], v["sq_aug"]

            # hcT: (128, NK, MCL) f16 via PE transposes + 2x DVE copies
            hcT = sb.tile([128, NK, MCL], F16, tag="hcT")
            for k in range(NK):
                trp = psT.tile([128, 1024], F16, tag="t")
                for t in range(NT):
                    nc.tensor.transpose(
                        trp[:, t * 128:(t + 1) * 128],
                        hc_nat[:, t, k * 128:(k + 1) * 128], ident[:])
                    if t == 3:
                        heater()
                nc.vector.tensor_copy(hcT[:, k, :], trp[:])

            # S^T: (65, 1024) fp32; row 64 = sc; rank-1 adds sq per row
            sT_s = sb.tile([MQL + 1, 2, 512], F16, tag="sT_s")
            for hf in range(2):
                sT_ps = ps.tile([MQL + 1, 512], F32, tag="sT")
                for k in range(NK):
                    nc.tensor.matmul(
                        sT_ps[:], stw[:, k, :],
                        hcT[:, k, hf * 512:(hf + 1) * 512],
                        start=(k == 0), stop=False)
                nc.tensor.matmul(sT_ps[:], sq_aug[:], ones_r[:],
                                 start=False, stop=True)
                nc.vector.tensor_copy(sT_s[:, hf, :], sT_ps[:])

            # sbank: per c-tile transpose -> (128, NT, 72) f16 (cols 0:65 used)
            sbank = ps.tile([128, NT, 72], F16, tag="sT")
            for t in range(NT):
                hf, j = divmod(t, 4)
                nc.tensor.transpose(
                    sbank[:, t, 0:MQL + 1],
                    sT_s[:, hf, j * 128:(j + 1) * 128],
                    ident[0:MQL + 1, 0:MQL + 1])
                if t == 4:
                    heater()
            v["sbank"] = sbank

            # ---- q2c chain (latency-critical, overlapped across batches):
            # mx -> score -> e2 -> U -> uT -> uB (PE broadcast) and the
            # rec2 scale, all issued inside s1 so four chains can be in
            # flight while later batches' S phases run.
            mx = sb.tile([128, NT], F32, tag="mx")
            nc.vector.tensor_reduce(mx[:], sbank[:, :, 0:MQL],
                                    axis=AX.X, op=ALU.max)
            score = sb.tile([128, NT], F32, tag="score")
            nc.vector.tensor_tensor(score[:], mx[:], sbank[:, :, MQL],
                                    op=ALU.add)
            E = sb.tile([128, NT, MQL], F16, tag="E")
            nc.scalar.activation(E[:], sbank[:, :, 0:MQL], AF.Exp,
                                 bias=bias_e[:], scale=1.0)
            v["E"] = E
            e2 = sb.tile([128, NT], F16, tag="e2")
            dsum = sb.tile([128, 1], F32, tag="dsum")
            nc.scalar.activation(e2[:], score[:], AF.Exp, bias=bias_0[:],
                                 scale=1.0, accum_out=dsum[:])
            U_ps = psU.tile([1, D], F32, tag="u")
            for t in range(NT):
                nc.tensor.matmul(U_ps[:], e2[:, t, None], hc_nat[:, t, :],
                                 start=(t == 0), stop=(t == NT - 1))
            uT = sb.tile([1, D], F16, tag="uT")
            nc.vector.tensor_copy(uT[:], U_ps[:])
            # uB: broadcast via K=1 matmul (gpsimd partition_broadcast is
            # ~7us effective — far too slow for the per-batch chain)
            uB_ps = psU.tile([128, D], F32, tag="u")
            nc.tensor.matmul(uB_ps[:], ones_r[0:1, 0:128], uT[:],
                             start=True, stop=True)
            uB = sb.tile([128, D], F16, tag="uB")
            nc.vector.tensor_copy(uB[:], uB_ps[:])
            v["uB"] = uB
            # rec2B = 1/sum(e2) on all partitions, via K=1 matmul broadcast
            dsum16 = sb.tile([128, 1], F16, tag="dsum16")
            nc.vector.tensor_copy(dsum16[:], dsum[:])
            den2_ps = psU.tile([1, 1], F32, tag="u")
            nc.tensor.matmul(den2_ps[:], dsum16[:], ones_col[:],
                             start=True, stop=True)
            den2_16 = sb.tile([1, 1], F16, tag="den2_16")
            nc.vector.tensor_copy(den2_16[:], den2_ps[:])
            den2B_ps = psU.tile([128, 1], F32, tag="u")
            nc.tensor.matmul(den2B_ps[:], ones_r[0:1, 0:128], den2_16[:],
                             start=True, stop=True)
            rec2B = sb.tile([128, 1], F32, tag="rec2B")
            nc.vector.reciprocal(rec2B[:], den2B_ps[:])
            v["rec2B"] = rec2B

        def s2(b):
            """gp products, c2q normalization, wT2, A matmuls + tanh(A)."""
            v = st[b]
            sbank, hc_nat, uB, E = v["sbank"], v["hc_nat"], v["uB"], v["E"]
            # c2q normalization first: En feeds wT2 -> A -> tanh(A), so it
            # must precede the products in the gp queue
            dens = sb.tile([128, NT], F32, tag="dens")
            nc.vector.tensor_reduce(dens[:], E[:], axis=AX.X, op=ALU.add)
            rec = sb.tile([128, NT], F32, tag="rec")
            nc.vector.reciprocal(rec[:], dens[:])
            En = sb.tile([128, NT, MQL], F16, tag="En")
            nc.vector.tensor_tensor(
                En[:], E[:], rec[:, :, None].broadcast_to((128, NT, MQL)),
                op=ALU.mult)

            # gp products Hc * U for the whole batch
            prods = []
            for h in range(2):
                prod = sb.tile([128, 4, 2 * D], F16, tag="prod", bufs=4)
                for i in range(2):
                    nc.gpsimd.tensor_tensor(
                        prod[:, i * 2:(i + 1) * 2, D:2 * D],
                        hc_nat[:, h * 4 + i * 2:h * 4 + (i + 1) * 2, :],
                        uB[:, None, :].broadcast_to((128, 2, D)), op=ALU.mult)
                prods.append(prod)
            v["prods"] = prods

            # wT2: paired En transposes; tile pair (2t, 2t+1) -> (128, 128)
            # rows 0:64 = q of even tile, 64:128 = q of odd tile
            wT2_ps = psA.tile([128, NT // 2, 128], F16, tag="A")
            for p in range(NT // 2):
                nc.tensor.transpose(
                    wT2_ps[:, p, :], En[:, 2 * p:2 * p + 2, :], ident[:])
                if p == 1:
                    heater()
            wT2 = sb.tile([128, NT // 2, 128], F16, tag="wT2")
            nc.vector.tensor_copy(wT2[:], wT2_ps[:])
            v["wT2"] = wT2

            # A matmuls (row-packed pairs) + tanh(A) + Hc*A: issued here so
            # the A-ring WARs resolve within this phase (tanh(A) follows each
            # pair immediately in the ACT stream).
            outs = []
            for h in range(2):
                out_t = sbo.tile([128, 4, 4 * D], F16, tag="out", bufs=4)
                outs.append(out_t)
                prod = prods[h]
                for i in range(2):
                    p = h * 2 + i          # tile pair (2p, 2p+1)
                    A_ps = psA.tile([128, 2, D], F32, tag="A")
                    nc.tensor.matmul(A_ps[:, 0, :], wT2[0:MQL, p, :],
                                     hq2[0:MQL, b, :], start=True, stop=True)
                    nc.tensor.matmul(A_ps[:, 1, :], wT2[MQL:128, p, :],
                                     hq2[MQL:128, b, :], start=True, stop=True)
                    nc.scalar.activation(out_t[:, i * 2:(i + 1) * 2, D:2 * D],
                                         A_ps[:], AF.Tanh, bias=bias_0[:],
                                         scale=1.0)
                    nc.vector.tensor_tensor(
                        prod[:, i * 2:(i + 1) * 2, 0:D], A_ps[:],
                        hc_nat[:, 2 * p:2 * p + 2, :], op=ALU.mult)
                    heater()
            v["outs"] = outs

        def s3(b):
            """remaining tanhs + stores (ACT + SP only)."""
            v = st[b]
            hc_nat, rec2B = v["hc_nat"], v["rec2B"]
            prods, outs = v["prods"], v["outs"]
            out_view = out_d[b].rearrange("(p t) j -> p t j", p=128)
            heater(3)
            for h in range(2):
                if b < BPC - 1:
                    qs = [slice(0, 4)]          # full half per store
                else:
                    qs = [slice(0, 2), slice(2, 4)]   # finer tail drain
                for q in qs:
                    tt = slice(h * 4 + q.start, h * 4 + q.stop)
                    nc.scalar.activation(outs[h][:, q, 0:D],
                                         hc_nat[:, tt, :],
                                         AF.Tanh, bias=bias_0[:], scale=1.0)
                    nc.scalar.activation(outs[h][:, q, 2 * D:3 * D],
                                         prods[h][:, q, 0:D], AF.Tanh,
                                         bias=bias_0[:], scale=1.0)
                    nc.scalar.activation(outs[h][:, q, 3 * D:4 * D],
                                         prods[h][:, q, D:2 * D], AF.Tanh,
                                         bias=bias_0[:], scale=rec2B[:])
                    nc.sync.dma_start(out_view[:, tt, :], outs[h][:, q, :])

        # software pipeline: one-batch lookahead so E/e2 of batch b+1 sit
        # before the tanh bulk of batch b in the ACT stream (the U matmuls
        # of b+1 must not wait for batch b's tanhs to drain).
        s1(0)
        s2(0)
        s1(1)
        s2(1)
        s3(0)
        s1(2)
        s2(2)
        s3(1)
        s1(3)
        s2(3)
        s3(2)
        s3(3)
    nc.compile()
    return nc


_NC = None


def _get_nc():
    global _NC
    if _NC is None:
        _NC = build_nc()
    return _NC


def run(inputs: dict, trace: bool = False, tmpdir: str | None = None):
    """Shard, run on 8 cores, gather. Returns (out, BassKernelResults)."""
    from concourse.bass_utils import run_bass_kernel_spmd

    if trace:
        # the axon NTFF hook module is absent in this image; inject it
        try:
            from antenv import axon_hooks  # noqa: F401
        except ImportError:
            import types
            import antenv
            from trn_agent_boot.trn_boot import _ntff_profile_via_ctypes
            mod = types.ModuleType("antenv.axon_hooks")
            _hook = _ntff_profile_via_ctypes('/opt/axon/libaxon_pjrt.so')
            mod.get_axon_ntff_profile_hook = lambda: _hook
            mod.set_axon_ntff_profile_hook = lambda h: None
            sys.modules["antenv.axon_hooks"] = mod
            antenv.axon_hooks = mod

    Hq = np.asarray(inputs["Hq"], dtype=np.float16)
    Hc = np.asarray(inputs["Hc"], dtype=np.float16)
    W = np.ascontiguousarray(np.asarray(inputs["W"], dtype=np.float32))
    IDM = np.eye(128, dtype=np.float16)
    nc = _get_nc()
    in_maps = [
        {"hq": np.ascontiguousarray(Hq[i * BPC:(i + 1) * BPC]),
         "hc": np.ascontiguousarray(Hc[i * BPC:(i + 1) * BPC]),
         "w": W, "idm": IDM}
        for i in range(NCORES)
    ]
    br = run_bass_kernel_spmd(nc, in_maps, list(range(NCORES)), trace=trace,
                              tmpdir=tmpdir)
    out = np.concatenate([br.results[i]["out"] for i in range(NCORES)],
                         axis=0).astype(np.float32)
    return out, br


def kernel(**inputs) -> np.ndarray:
    out, _ = run(inputs, trace=False)
    return out
